# revision 19
# baseline (speedup 1.0000x reference)
"""Biaffine kernel for Trainium2, data-parallel over batch on 8 NeuronCores.

Problem: inputs [8,512,768] f32, weight1 [768,12,768], weight2 [1537,12],
mask [8,512] i32 -> logits [8,12,512,512] f32 (see reference).

Key trick: masked outputs are -1e12 (or -2e12), and f32 addition absorbs any
|v| < half-ulp(1e12) = 32768. Raw logits are |v| <~ 1e3, so we never mask the
matmul inputs: out = raw + C with C in {0, -1e12, -2e12} reproduces the
reference bit-exactly on masked entries and exactly on unmasked ones.

Per core (batch b):
  XT[i,x]    = X[x,i]                             (PE transpose)
  tmpT[j,x]  = sum_i W1[i,o,j] * XT[i,x]          (matmul1, per o)
  raw[x,y]   = sum_j tmpT[j,x] * XT[j,y]          (matmul2, per o)
               + linjT[o,y]                       (K=24 selector matmul)
  out[o,x,y] = (raw + linT[o,x]) + C[x,y]         (one DVE scalar_tensor_tensor)
where linT[o,x] = lin_i[x,o]+bias[o], linjT[o,y] = lin_j[y,o],
C = NEG*(m outer m) + C0,  C0 = -NEG*tril(k=-1) - NEG.
"""

import numpy as np

import concourse.bass as bass
import concourse.mybir as mybir
import concourse.tile as tile
from concourse import bacc
from concourse.bass_utils import run_bass_kernel_spmd

B, L, H, O = 8, 512, 768, 12
NEG = 1e12
F32 = mybir.dt.float32
F32R = mybir.dt.float32r
NCORES = 8

_cached_nc = None


def build_nc():
    nc = bacc.Bacc(None, target_bir_lowering=False)

    x_d = nc.dram_tensor("x", [L, H], F32R, kind="ExternalInput")
    w1_d = nc.dram_tensor("w1", [H, O, H], F32R, kind="ExternalInput")
    w2a_d = nc.dram_tensor("w2a", [H + 1, 128], F32R, kind="ExternalInput")
    selo_d = nc.dram_tensor("selo", [128, O * 128], F32R, kind="ExternalInput")
    mrow_d = nc.dram_tensor("mrow", [1, L], F32R, kind="ExternalInput")
    ones_d = nc.dram_tensor("ones1", [1, L], F32R, kind="ExternalInput")
    ident_d = nc.dram_tensor("ident", [128, 128], F32R, kind="ExternalInput")
    c0_d = nc.dram_tensor("c0", [L, L], F32, kind="ExternalInput")
    out_d = nc.dram_tensor("out", [O, L, L], F32, kind="ExternalOutput")

    KT = H // 128   # 6 k-tiles over i/j
    XC = L // 128   # 4 x-chunks

    with tile.TileContext(nc) as tc:
        with (
            tc.tile_pool(name="const", bufs=1) as cpool,
            tc.tile_pool(name="work", bufs=1) as wpool,
            tc.tile_pool(name="w1p", bufs=3) as w1pool,
            tc.tile_pool(name="tmpp", bufs=2) as tmppool,
            tc.tile_pool(name="outp", bufs=4) as outpool,
        ):
            # ident + mrow + X on the sync queue (transposes/pm gate the start);
            # W1[o=0] prefetch + selo + csb on the scalar HWDGE queue.
            ident = cpool.tile([128, 128], F32R, tag="ident")
            nc.sync.dma_start(ident[:], ident_d[:])
            mrow = cpool.tile([1, L], F32R, tag="mrow")
            nc.sync.dma_start(mrow[:], mrow_d[:])
            xnat = []
            for c in range(XC):
                xn = wpool.tile([128, H], F32R, tag=f"xnat{c}")
                nc.sync.dma_start(xn[:], x_d[c * 128 : (c + 1) * 128, :])
                xnat.append(xn)
            ones1 = cpool.tile([1, L], F32R, tag="ones1")
            nc.sync.dma_start(ones1[:], ones_d[:])
            w2sb = cpool.tile([128, KT * 128], F32R, tag="w2sb")
            for kt in range(KT):
                nc.sync.dma_start(
                    w2sb[:, kt * 128 : (kt + 1) * 128],
                    w2a_d[kt * 128 : (kt + 1) * 128, :],
                )
            w2last = cpool.tile([1, 128], F32R, tag="w2last")
            nc.sync.dma_start(w2last[:], w2a_d[H : H + 1, :])

            csb = []
            for c in range(XC):
                cs = cpool.tile([128, L], F32, tag=f"csb{c}")
                nc.sync.dma_start(cs[:], c0_d[c * 128 : (c + 1) * 128, :])
                csb.append(cs)
            selo = cpool.tile([128, O * 128], F32R, tag="selo")
            nc.scalar.dma_start(selo[:], selo_d[:])
            w1t_next = [w1pool.tile([128, H], F32R, tag=f"w1t{kt}", name=f"w1ta{kt}") for kt in range(KT)]
            for kt in range(KT):
                nc.scalar.dma_start(
                    w1t_next[kt], w1_d[kt * 128 : (kt + 1) * 128, 0, :]
                )
            w1t_next2 = [w1pool.tile([128, H], F32R, tag=f"w1t{kt}", name=f"w1tb{kt}") for kt in range(KT)]
            for kt in range(KT):
                nc.scalar.dma_start(
                    w1t_next2[kt][:], w1_d[kt * 128 : (kt + 1) * 128, 1, :]
                )

            xt_all = wpool.tile([128, KT * L], F32R, tag="xt")
            xt = [xt_all[:, kt * L : (kt + 1) * L] for kt in range(KT)]
            augall = wpool.tile([128, L], F32R, tag="augall")
            linTT = wpool.tile([128, XC * O], F32, tag="linTT")
            with tc.tile_pool(name="pspro", bufs=1, space="PSUM") as pspro:
                # --- HAM warmup: dummy transposes on ident (first DMA) ---
                for w in range(16):
                    wp = pspro.tile([128, 128], F32R, tag="warm", bufs=1, name=f"warm{w}")
                    nc.tensor.transpose(wp[:], ident[:], ident[:])

                # --- C map: csb[c] <- NEG * (mx outer my) + csb[c] ---
                for c in range(XC):
                    pm = pspro.tile([128, L], F32, tag="pm", bufs=1)
                    nc.tensor.matmul(
                        pm[:],
                        mrow[:, c * 128 : (c + 1) * 128],
                        mrow[:],
                        start=True,
                        stop=True,
                    )
                    nc.vector.scalar_tensor_tensor(
                        out=csb[c][:],
                        in0=pm[:],
                        scalar=NEG,
                        in1=csb[c][:],
                        op0=mybir.AluOpType.mult,
                        op1=mybir.AluOpType.add,
                    )

                # --- X transpose to XT ---
                for c in range(XC):
                    for kt in range(KT):
                        tp = pspro.tile([128, 128], F32R, tag="tp", bufs=2)
                        nc.tensor.transpose(
                            tp[:],
                            xnat[c][:, kt * 128 : (kt + 1) * 128],
                            ident[:],
                        )
                        nc.vector.tensor_copy(
                            xt[kt][:, c * 128 : (c + 1) * 128], tp[:]
                        )

                # --- augall: rows o = linT'[o], rows 12+o = linjT'[o], rest 0
                pa = pspro.tile([128, L], F32, tag="pa", bufs=1)
                for kt in range(KT):
                    nc.tensor.matmul(
                        pa[:],
                        w2sb[:, kt * 128 : (kt + 1) * 128],
                        xt[kt],
                        start=(kt == 0),
                        stop=False,
                    )
                nc.tensor.matmul(
                    pa[:], w2last[:], ones1[:], start=False, stop=True
                )
                nc.vector.tensor_copy(augall[:], pa[:])

                # --- linTT [128, XC*O]: transpose of augall rows 0..11 ---
                for c in range(XC):
                    pt = pspro.tile([128, O], F32R, tag="pt", bufs=2)
                    nc.tensor.transpose(
                        pt[:],
                        augall[0:O, c * 128 : (c + 1) * 128],
                        ident[0:O, 0:O],
                    )
                    nc.vector.tensor_copy(linTT[:, c * O : (c + 1) * O], pt[:])

            # --- main loop over labels ---
            with tc.tile_pool(name="psmain", bufs=1, space="PSUM") as psm:
                for o in range(O):
                    w1t = w1t_next
                    w1t_next = w1t_next2
                    if o + 2 < O:
                        w1t_next2 = [
                            w1pool.tile([128, H], F32R, tag=f"w1t{kt}", name=f"w1t_{o}_{kt}")
                            for kt in range(KT)
                        ]
                        for kt in range(KT):
                            nc.sync.dma_start(
                                w1t_next2[kt][:],
                                w1_d[kt * 128 : (kt + 1) * 128, o + 2, :],
                            )

                    # matmul1: tmpT[j, x] (6 m-chunks x 6 k-tiles)
                    tmp_all = tmppool.tile([128, KT * L], F32R, tag="tmp", name=f"tmp_{o}")
                    tmp = [tmp_all[:, m * L : (m + 1) * L] for m in range(KT)]
                    for m in range(KT):
                        p1 = psm.tile([128, L], F32, tag="t1", bufs=3)
                        for kt in range(KT):
                            nc.tensor.matmul(
                                p1[:],
                                w1t[kt][:, m * 128 : (m + 1) * 128],
                                xt[kt],
                                start=(kt == 0),
                                stop=(kt == KT - 1),
                            )
                        nc.vector.tensor_copy(tmp[m], p1[:])

                    # matmul2 + linj aug + epilogue per x-chunk
                    for c in range(XC):
                        p2 = psm.tile([128, L], F32, tag="t2", bufs=5)
                        for jr in range(KT):
                            nc.tensor.matmul(
                                p2[:],
                                tmp[jr][:, c * 128 : (c + 1) * 128],
                                xt[jr],
                                start=(jr == 0),
                                stop=False,
                            )
                        nc.tensor.matmul(
                            p2[:],
                            selo[:, o * 128 : (o + 1) * 128],
                            augall[:],
                            start=False,
                            stop=True,
                        )
                        osb = outpool.tile([128, L], F32, tag="osb")
                        nc.vector.scalar_tensor_tensor(
                            out=osb[:],
                            in0=p2[:],
                            scalar=linTT[:, c * O + o : c * O + o + 1],
                            in1=csb[c][:],
                            op0=mybir.AluOpType.add,
                            op1=mybir.AluOpType.add,
                        )
                        nc.scalar.dma_start(
                            out_d[o, c * 128 : (c + 1) * 128, :], osb[:]
                        )

    nc.compile()
    return nc


def _get_nc():
    global _cached_nc
    if _cached_nc is None:
        _cached_nc = build_nc()
    return _cached_nc


def _host_consts(weight2):
    w2a = np.zeros((H + 1, 128), dtype=np.float32)
    # cols o: linT' = lin_i + bias; cols O+o: linjT' = lin_j
    w2a[:H, :O] = weight2[:H, :]
    w2a[H, :O] = weight2[2 * H, :]
    w2a[:H, O : 2 * O] = weight2[H : 2 * H, :]
    selo = np.zeros((128, O * 128), dtype=np.float32)
    for o in range(O):
        selo[O + o, o * 128 : (o + 1) * 128] = 1.0
    ident = np.eye(128, dtype=np.float32)
    ones1 = np.ones((1, L), dtype=np.float32)
    tril = np.tril(np.ones((L, L), dtype=np.float32), k=-1)
    c0 = (-NEG * tril - NEG).astype(np.float32)
    return w2a, selo, ident, ones1, c0


def _run(inputs, weight1, weight2, mask, trace=False):
    nc = _get_nc()
    w2a, selo, ident, ones1, c0 = _host_consts(np.asarray(weight2, dtype=np.float32))
    w1 = np.ascontiguousarray(np.asarray(weight1, dtype=np.float32))
    in_maps = []
    for b in range(NCORES):
        m = np.asarray(mask[b], dtype=np.float32)
        in_maps.append(
            {
                "x": np.ascontiguousarray(np.asarray(inputs[b], dtype=np.float32)),
                "w1": w1,
                "w2a": w2a,
                "selo": selo,
                "mrow": np.ascontiguousarray(m[None, :]),
                "ones1": ones1,
                "ident": ident,
                "c0": c0,
            }
        )
    try:
        br = run_bass_kernel_spmd(
            nc, in_maps, core_ids=list(range(NCORES)), trace=trace
        )
        out = np.stack([br.results[b]["out"] for b in range(NCORES)], axis=0)
        return out, br
    except Exception:  # noqa: BLE001
        # Neuron devices occasionally come up wedged from a previous process
        # (NRT_EXEC_UNIT_UNRECOVERABLE). A wedged device recovers on the next
        # fresh process, so retry execution in clean subprocesses.
        out = _run_in_subprocess(in_maps)
        return out, None


def _run_in_subprocess(in_maps):
    import os
    import subprocess
    import sys
    import tempfile
    import time

    d = tempfile.mkdtemp(prefix="biaffine_kernel_")
    inp = os.path.join(d, "in.npz")
    outp = os.path.join(d, "out.npy")
    flat = {}
    for b in range(NCORES):
        for k, v in in_maps[b].items():
            flat[f"{k}__{b}"] = v
    np.savez(inp, **flat)
    runner = os.path.join(d, "runner.py")
    with open(runner, "w") as f:
        f.write(
            f"""
import sys
sys.path.insert(0, {os.path.dirname(os.path.abspath(__file__))!r})
import numpy as np
import kernel
d = np.load({inp!r})
in_maps = [dict() for _ in range(kernel.NCORES)]
for key in d.files:
    k, b = key.rsplit('__', 1)
    in_maps[int(b)][k] = d[key]
nc = kernel._get_nc()
from concourse.bass_utils import run_bass_kernel_spmd
br = run_bass_kernel_spmd(nc, in_maps, core_ids=list(range(kernel.NCORES)), trace=False)
np.save({outp!r}, np.stack([br.results[b]["out"] for b in range(kernel.NCORES)], axis=0))
"""
        )
    last = None
    for attempt in range(4):
        r = subprocess.run(
            [sys.executable, runner], capture_output=True, timeout=1200
        )
        if r.returncode == 0 and os.path.exists(outp):
            return np.load(outp)
        last = r
        time.sleep(5.0)
    raise RuntimeError(
        "device execution failed after retries: "
        + (last.stderr.decode(errors="replace")[-2000:] if last else "")
    )


def kernel(inputs, weight1, weight2, mask):
    out, _ = _run(inputs, weight1, weight2, mask)
    return out


# revision 20
# speedup vs baseline: 1.0040x; 1.0040x over previous
"""Biaffine kernel for Trainium2, data-parallel over batch on 8 NeuronCores.

Problem: inputs [8,512,768] f32, weight1 [768,12,768], weight2 [1537,12],
mask [8,512] i32 -> logits [8,12,512,512] f32 (see reference).

Key trick: masked outputs are -1e12 (or -2e12), and f32 addition absorbs any
|v| < half-ulp(1e12) = 32768. Raw logits are |v| <~ 1e3, so we never mask the
matmul inputs: out = raw + C with C in {0, -1e12, -2e12} reproduces the
reference bit-exactly on masked entries and exactly on unmasked ones.

Per core (batch b):
  XT[i,x]    = X[x,i]                             (PE transpose)
  tmpT[j,x]  = sum_i W1[i,o,j] * XT[i,x]          (matmul1, per o)
  raw[x,y]   = sum_j tmpT[j,x] * XT[j,y]          (matmul2, per o)
               + linjT[o,y]                       (K=24 selector matmul)
  out[o,x,y] = (raw + linT[o,x]) + C[x,y]         (one DVE scalar_tensor_tensor)
where linT[o,x] = lin_i[x,o]+bias[o], linjT[o,y] = lin_j[y,o],
C = NEG*(m outer m) + C0,  C0 = -NEG*tril(k=-1) - NEG.
"""

import numpy as np

import concourse.bass as bass
import concourse.mybir as mybir
import concourse.tile as tile
from concourse import bacc
from concourse.bass_utils import run_bass_kernel_spmd

B, L, H, O = 8, 512, 768, 12
NEG = 1e12
F32 = mybir.dt.float32
F32R = mybir.dt.float32r
NCORES = 8

_cached_nc = None


def build_nc():
    nc = bacc.Bacc(None, target_bir_lowering=False)

    x_d = nc.dram_tensor("x", [L, H], F32R, kind="ExternalInput")
    w1_d = nc.dram_tensor("w1", [H, O, H], F32R, kind="ExternalInput")
    w2a_d = nc.dram_tensor("w2a", [H + 1, 128], F32R, kind="ExternalInput")
    selo_d = nc.dram_tensor("selo", [128, O * 128], F32R, kind="ExternalInput")
    mrow_d = nc.dram_tensor("mrow", [1, L], F32R, kind="ExternalInput")
    ones_d = nc.dram_tensor("ones1", [1, L], F32R, kind="ExternalInput")
    ident_d = nc.dram_tensor("ident", [128, 128], F32R, kind="ExternalInput")
    c0_d = nc.dram_tensor("c0", [L, L], F32, kind="ExternalInput")
    out_d = nc.dram_tensor("out", [O, L, L], F32, kind="ExternalOutput")

    KT = H // 128   # 6 k-tiles over i/j
    XC = L // 128   # 4 x-chunks

    with tile.TileContext(nc) as tc:
        with (
            tc.tile_pool(name="const", bufs=1) as cpool,
            tc.tile_pool(name="work", bufs=1) as wpool,
            tc.tile_pool(name="w1p", bufs=3) as w1pool,
            tc.tile_pool(name="tmpp", bufs=2) as tmppool,
            tc.tile_pool(name="outp", bufs=6) as outpool,
        ):
            # ident + mrow + X on the sync queue (transposes/pm gate the start);
            # W1[o=0] prefetch + selo + csb on the scalar HWDGE queue.
            ident = cpool.tile([128, 128], F32R, tag="ident")
            nc.sync.dma_start(ident[:], ident_d[:])
            mrow = cpool.tile([1, L], F32R, tag="mrow")
            nc.sync.dma_start(mrow[:], mrow_d[:])
            xnat = []
            for c in range(XC):
                xn = wpool.tile([128, H], F32R, tag=f"xnat{c}")
                nc.sync.dma_start(xn[:], x_d[c * 128 : (c + 1) * 128, :])
                xnat.append(xn)
            ones1 = cpool.tile([1, L], F32R, tag="ones1")
            nc.sync.dma_start(ones1[:], ones_d[:])
            w2sb = cpool.tile([128, KT * 128], F32R, tag="w2sb")
            for kt in range(KT):
                nc.sync.dma_start(
                    w2sb[:, kt * 128 : (kt + 1) * 128],
                    w2a_d[kt * 128 : (kt + 1) * 128, :],
                )
            w2last = cpool.tile([1, 128], F32R, tag="w2last")
            nc.sync.dma_start(w2last[:], w2a_d[H : H + 1, :])

            csb = []
            for c in range(XC):
                cs = cpool.tile([128, L], F32, tag=f"csb{c}")
                nc.sync.dma_start(cs[:], c0_d[c * 128 : (c + 1) * 128, :])
                csb.append(cs)
            selo = cpool.tile([128, O * 128], F32R, tag="selo")
            nc.scalar.dma_start(selo[:], selo_d[:])
            w1t_next = [w1pool.tile([128, H], F32R, tag=f"w1t{kt}", name=f"w1ta{kt}") for kt in range(KT)]
            for kt in range(KT):
                nc.scalar.dma_start(
                    w1t_next[kt], w1_d[kt * 128 : (kt + 1) * 128, 0, :]
                )
            w1t_next2 = [w1pool.tile([128, H], F32R, tag=f"w1t{kt}", name=f"w1tb{kt}") for kt in range(KT)]
            for kt in range(KT):
                nc.scalar.dma_start(
                    w1t_next2[kt][:], w1_d[kt * 128 : (kt + 1) * 128, 1, :]
                )

            xt_all = wpool.tile([128, KT * L], F32R, tag="xt")
            xt = [xt_all[:, kt * L : (kt + 1) * L] for kt in range(KT)]
            augall = wpool.tile([128, L], F32R, tag="augall")
            linTT = wpool.tile([128, XC * O], F32, tag="linTT")
            with tc.tile_pool(name="pspro", bufs=1, space="PSUM") as pspro:
                # --- C map: csb[c] <- NEG * (mx outer my) + csb[c] ---
                for c in range(XC):
                    pm = pspro.tile([128, L], F32, tag="pm", bufs=2)
                    nc.tensor.matmul(
                        pm[:],
                        mrow[:, c * 128 : (c + 1) * 128],
                        mrow[:],
                        start=True,
                        stop=True,
                    )
                    nc.vector.scalar_tensor_tensor(
                        out=csb[c][:],
                        in0=pm[:],
                        scalar=NEG,
                        in1=csb[c][:],
                        op0=mybir.AluOpType.mult,
                        op1=mybir.AluOpType.add,
                    )

                # --- X transpose to XT ---
                for c in range(XC):
                    for kt in range(KT):
                        tp = pspro.tile([128, 128], F32R, tag="tp", bufs=2)
                        nc.tensor.transpose(
                            tp[:],
                            xnat[c][:, kt * 128 : (kt + 1) * 128],
                            ident[:],
                        )
                        nc.vector.tensor_copy(
                            xt[kt][:, c * 128 : (c + 1) * 128], tp[:]
                        )

                # --- augall: rows o = linT'[o], rows 12+o = linjT'[o], rest 0
                pa = pspro.tile([128, L], F32, tag="pa", bufs=1)
                for kt in range(KT):
                    nc.tensor.matmul(
                        pa[:],
                        w2sb[:, kt * 128 : (kt + 1) * 128],
                        xt[kt],
                        start=(kt == 0),
                        stop=False,
                    )
                nc.tensor.matmul(
                    pa[:], w2last[:], ones1[:], start=False, stop=True
                )
                nc.vector.tensor_copy(augall[:], pa[:])

                # --- linTT [128, XC*O]: transpose of augall rows 0..11 ---
                for c in range(XC):
                    pt = pspro.tile([128, O], F32R, tag="pt", bufs=2)
                    nc.tensor.transpose(
                        pt[:],
                        augall[0:O, c * 128 : (c + 1) * 128],
                        ident[0:O, 0:O],
                    )
                    nc.vector.tensor_copy(linTT[:, c * O : (c + 1) * O], pt[:])

            # --- main loop over labels ---
            with tc.tile_pool(name="psmain", bufs=1, space="PSUM") as psm:
                for o in range(O):
                    w1t = w1t_next
                    w1t_next = w1t_next2
                    if o + 2 < O:
                        w1t_next2 = [
                            w1pool.tile([128, H], F32R, tag=f"w1t{kt}", name=f"w1t_{o}_{kt}")
                            for kt in range(KT)
                        ]
                        for kt in range(KT):
                            nc.sync.dma_start(
                                w1t_next2[kt][:],
                                w1_d[kt * 128 : (kt + 1) * 128, o + 2, :],
                            )

                    # matmul1: tmpT[j, x] (6 m-chunks x 6 k-tiles)
                    tmp_all = tmppool.tile([128, KT * L], F32R, tag="tmp", name=f"tmp_{o}")
                    tmp = [tmp_all[:, m * L : (m + 1) * L] for m in range(KT)]
                    for m in range(KT):
                        p1 = psm.tile([128, L], F32, tag="t1", bufs=2)
                        for kt in range(KT):
                            nc.tensor.matmul(
                                p1[:],
                                w1t[kt][:, m * 128 : (m + 1) * 128],
                                xt[kt],
                                start=(kt == 0),
                                stop=(kt == KT - 1),
                            )
                        nc.vector.tensor_copy(tmp[m], p1[:])

                    # matmul2 + linj aug + epilogue per x-chunk
                    for c in range(XC):
                        p2 = psm.tile([128, L], F32, tag="t2", bufs=6)
                        for jr in range(KT):
                            nc.tensor.matmul(
                                p2[:],
                                tmp[jr][:, c * 128 : (c + 1) * 128],
                                xt[jr],
                                start=(jr == 0),
                                stop=False,
                            )
                        nc.tensor.matmul(
                            p2[:],
                            selo[:, o * 128 : (o + 1) * 128],
                            augall[:],
                            start=False,
                            stop=True,
                        )
                        osb = outpool.tile([128, L], F32, tag="osb")
                        nc.vector.scalar_tensor_tensor(
                            out=osb[:],
                            in0=p2[:],
                            scalar=linTT[:, c * O + o : c * O + o + 1],
                            in1=csb[c][:],
                            op0=mybir.AluOpType.add,
                            op1=mybir.AluOpType.add,
                        )
                        nc.scalar.dma_start(
                            out_d[o, c * 128 : (c + 1) * 128, :], osb[:]
                        )

    nc.compile()
    return nc


def _get_nc():
    global _cached_nc
    if _cached_nc is None:
        _cached_nc = build_nc()
    return _cached_nc


def _host_consts(weight2):
    w2a = np.zeros((H + 1, 128), dtype=np.float32)
    # cols o: linT' = lin_i + bias; cols O+o: linjT' = lin_j
    w2a[:H, :O] = weight2[:H, :]
    w2a[H, :O] = weight2[2 * H, :]
    w2a[:H, O : 2 * O] = weight2[H : 2 * H, :]
    selo = np.zeros((128, O * 128), dtype=np.float32)
    for o in range(O):
        selo[O + o, o * 128 : (o + 1) * 128] = 1.0
    ident = np.eye(128, dtype=np.float32)
    ones1 = np.ones((1, L), dtype=np.float32)
    tril = np.tril(np.ones((L, L), dtype=np.float32), k=-1)
    c0 = (-NEG * tril - NEG).astype(np.float32)
    return w2a, selo, ident, ones1, c0


def _run(inputs, weight1, weight2, mask, trace=False):
    nc = _get_nc()
    w2a, selo, ident, ones1, c0 = _host_consts(np.asarray(weight2, dtype=np.float32))
    w1 = np.ascontiguousarray(np.asarray(weight1, dtype=np.float32))
    in_maps = []
    for b in range(NCORES):
        m = np.asarray(mask[b], dtype=np.float32)
        in_maps.append(
            {
                "x": np.ascontiguousarray(np.asarray(inputs[b], dtype=np.float32)),
                "w1": w1,
                "w2a": w2a,
                "selo": selo,
                "mrow": np.ascontiguousarray(m[None, :]),
                "ones1": ones1,
                "ident": ident,
                "c0": c0,
            }
        )
    try:
        br = run_bass_kernel_spmd(
            nc, in_maps, core_ids=list(range(NCORES)), trace=trace
        )
        out = np.stack([br.results[b]["out"] for b in range(NCORES)], axis=0)
        return out, br
    except Exception:  # noqa: BLE001
        # Neuron devices occasionally come up wedged from a previous process
        # (NRT_EXEC_UNIT_UNRECOVERABLE). A wedged device recovers on the next
        # fresh process, so retry execution in clean subprocesses.
        out = _run_in_subprocess(in_maps)
        return out, None


def _run_in_subprocess(in_maps):
    import os
    import subprocess
    import sys
    import tempfile
    import time

    d = tempfile.mkdtemp(prefix="biaffine_kernel_")
    inp = os.path.join(d, "in.npz")
    outp = os.path.join(d, "out.npy")
    flat = {}
    for b in range(NCORES):
        for k, v in in_maps[b].items():
            flat[f"{k}__{b}"] = v
    np.savez(inp, **flat)
    runner = os.path.join(d, "runner.py")
    with open(runner, "w") as f:
        f.write(
            f"""
import sys
sys.path.insert(0, {os.path.dirname(os.path.abspath(__file__))!r})
import numpy as np
import kernel
d = np.load({inp!r})
in_maps = [dict() for _ in range(kernel.NCORES)]
for key in d.files:
    k, b = key.rsplit('__', 1)
    in_maps[int(b)][k] = d[key]
nc = kernel._get_nc()
from concourse.bass_utils import run_bass_kernel_spmd
br = run_bass_kernel_spmd(nc, in_maps, core_ids=list(range(kernel.NCORES)), trace=False)
np.save({outp!r}, np.stack([br.results[b]["out"] for b in range(kernel.NCORES)], axis=0))
"""
        )
    last = None
    for attempt in range(4):
        r = subprocess.run(
            [sys.executable, runner], capture_output=True, timeout=1200
        )
        if r.returncode == 0 and os.path.exists(outp):
            return np.load(outp)
        last = r
        time.sleep(5.0)
    raise RuntimeError(
        "device execution failed after retries: "
        + (last.stderr.decode(errors="replace")[-2000:] if last else "")
    )


def kernel(inputs, weight1, weight2, mask):
    out, _ = _run(inputs, weight1, weight2, mask)
    return out


# revision 21
# speedup vs baseline: 1.0227x; 1.0187x over previous
"""Biaffine kernel for Trainium2, data-parallel over batch on 8 NeuronCores.

Problem: inputs [8,512,768] f32, weight1 [768,12,768], weight2 [1537,12],
mask [8,512] i32 -> logits [8,12,512,512] f32 (see reference).

Key trick: masked outputs are -1e12 (or -2e12), and f32 addition absorbs any
|v| < half-ulp(1e12) = 32768. Raw logits are |v| <~ 1e3, so we never mask the
matmul inputs: out = raw + C with C in {0, -1e12, -2e12} reproduces the
reference bit-exactly on masked entries and exactly on unmasked ones.

Per core (batch b):
  XT[i,x]    = X[x,i]                             (PE transpose)
  tmpT[j,x]  = sum_i W1[i,o,j] * XT[i,x]          (matmul1, per o)
  raw[x,y]   = sum_j tmpT[j,x] * XT[j,y]          (matmul2, per o)
               + linjT[o,y]                       (K=24 selector matmul)
  out[o,x,y] = (raw + linT[o,x]) + C[x,y]         (one DVE scalar_tensor_tensor)
where linT[o,x] = lin_i[x,o]+bias[o], linjT[o,y] = lin_j[y,o],
C = NEG*(m outer m) + C0,  C0 = -NEG*tril(k=-1) - NEG.
"""

import numpy as np

import concourse.bass as bass
import concourse.mybir as mybir
import concourse.tile as tile
from concourse import bacc
from concourse.bass_utils import run_bass_kernel_spmd

B, L, H, O = 8, 512, 768, 12
NEG = 1e12
F32 = mybir.dt.float32
F32R = mybir.dt.float32r
NCORES = 8

_cached_nc = None


def build_nc():
    nc = bacc.Bacc(None, target_bir_lowering=False)

    x_d = nc.dram_tensor("x", [L, H], F32R, kind="ExternalInput")
    w1_d = nc.dram_tensor("w1", [H, O, H], F32R, kind="ExternalInput")
    w2a_d = nc.dram_tensor("w2a", [H + 1, 128], F32R, kind="ExternalInput")
    selo_d = nc.dram_tensor("selo", [128, O * 128], F32R, kind="ExternalInput")
    mrow_d = nc.dram_tensor("mrow", [1, L], F32R, kind="ExternalInput")
    ones_d = nc.dram_tensor("ones1", [1, L], F32R, kind="ExternalInput")
    ident_d = nc.dram_tensor("ident", [128, 128], F32R, kind="ExternalInput")
    c0_d = nc.dram_tensor("c0", [L, L], F32, kind="ExternalInput")
    out_d = nc.dram_tensor("out", [O, L, L], F32, kind="ExternalOutput")

    KT = H // 128   # 6 k-tiles over i/j
    XC = L // 128   # 4 x-chunks

    with tile.TileContext(nc) as tc:
        with (
            tc.tile_pool(name="const", bufs=1) as cpool,
            tc.tile_pool(name="work", bufs=1) as wpool,
            tc.tile_pool(name="w1p", bufs=3) as w1pool,
            tc.tile_pool(name="tmpp", bufs=3) as tmppool,
            tc.tile_pool(name="outp", bufs=6) as outpool,
        ):
            # ident + mrow + X on the sync queue (transposes/pm gate the start);
            # W1[o=0] prefetch + selo + csb on the scalar HWDGE queue.
            ident = cpool.tile([128, 128], F32R, tag="ident")
            nc.sync.dma_start(ident[:], ident_d[:])
            mrow = cpool.tile([1, L], F32R, tag="mrow")
            nc.sync.dma_start(mrow[:], mrow_d[:])
            xnat = []
            for c in range(XC):
                xn = wpool.tile([128, H], F32R, tag=f"xnat{c}")
                nc.sync.dma_start(xn[:], x_d[c * 128 : (c + 1) * 128, :])
                xnat.append(xn)
            ones1 = cpool.tile([1, L], F32R, tag="ones1")
            nc.sync.dma_start(ones1[:], ones_d[:])
            w2sb = cpool.tile([128, KT * 128], F32R, tag="w2sb")
            for kt in range(KT):
                nc.sync.dma_start(
                    w2sb[:, kt * 128 : (kt + 1) * 128],
                    w2a_d[kt * 128 : (kt + 1) * 128, :],
                )
            w2last = cpool.tile([1, 128], F32R, tag="w2last")
            nc.sync.dma_start(w2last[:], w2a_d[H : H + 1, :])

            csb = []
            for c in range(XC):
                cs = cpool.tile([128, L], F32, tag=f"csb{c}")
                nc.sync.dma_start(cs[:], c0_d[c * 128 : (c + 1) * 128, :])
                csb.append(cs)
            selo = cpool.tile([128, O * 128], F32R, tag="selo")
            nc.scalar.dma_start(selo[:], selo_d[:])
            w1t_next = [w1pool.tile([128, H], F32R, tag=f"w1t{kt}", name=f"w1ta{kt}") for kt in range(KT)]
            for kt in range(KT):
                nc.scalar.dma_start(
                    w1t_next[kt], w1_d[kt * 128 : (kt + 1) * 128, 0, :]
                )
            w1t_next2 = [w1pool.tile([128, H], F32R, tag=f"w1t{kt}", name=f"w1tb{kt}") for kt in range(KT)]
            for kt in range(KT):
                nc.scalar.dma_start(
                    w1t_next2[kt][:], w1_d[kt * 128 : (kt + 1) * 128, 1, :]
                )

            xt_all = wpool.tile([128, KT * L], F32R, tag="xt")
            xt = [xt_all[:, kt * L : (kt + 1) * L] for kt in range(KT)]
            augall = wpool.tile([128, L], F32R, tag="augall")
            linTT = wpool.tile([128, XC * O], F32, tag="linTT")
            with tc.tile_pool(name="pspro", bufs=1, space="PSUM") as pspro:
                # --- C map: csb[c] <- NEG * (mx outer my) + csb[c] ---
                for c in range(XC):
                    pm = pspro.tile([128, L], F32, tag="pm", bufs=2)
                    nc.tensor.matmul(
                        pm[:],
                        mrow[:, c * 128 : (c + 1) * 128],
                        mrow[:],
                        start=True,
                        stop=True,
                    )
                    nc.vector.scalar_tensor_tensor(
                        out=csb[c][:],
                        in0=pm[:],
                        scalar=NEG,
                        in1=csb[c][:],
                        op0=mybir.AluOpType.mult,
                        op1=mybir.AluOpType.add,
                    )

                # --- X transpose to XT ---
                for c in range(XC):
                    for kt in range(KT):
                        tp = pspro.tile([128, 128], F32R, tag="tp", bufs=2)
                        nc.tensor.transpose(
                            tp[:],
                            xnat[c][:, kt * 128 : (kt + 1) * 128],
                            ident[:],
                        )
                        nc.vector.tensor_copy(
                            xt[kt][:, c * 128 : (c + 1) * 128], tp[:]
                        )

                # --- augall: rows o = linT'[o], rows 12+o = linjT'[o], rest 0
                pa = pspro.tile([128, L], F32, tag="pa", bufs=1)
                for kt in range(KT):
                    nc.tensor.matmul(
                        pa[:],
                        w2sb[:, kt * 128 : (kt + 1) * 128],
                        xt[kt],
                        start=(kt == 0),
                        stop=False,
                    )
                nc.tensor.matmul(
                    pa[:], w2last[:], ones1[:], start=False, stop=True
                )
                nc.vector.tensor_copy(augall[:], pa[:])

                # --- linTT [128, XC*O]: transpose of augall rows 0..11 ---
                for c in range(XC):
                    pt = pspro.tile([128, O], F32R, tag="pt", bufs=2)
                    nc.tensor.transpose(
                        pt[:],
                        augall[0:O, c * 128 : (c + 1) * 128],
                        ident[0:O, 0:O],
                    )
                    nc.vector.tensor_copy(linTT[:, c * O : (c + 1) * O], pt[:])

            # --- main loop over labels ---
            with tc.tile_pool(name="psmain", bufs=1, space="PSUM") as psm:
                for o in range(O):
                    w1t = w1t_next
                    w1t_next = w1t_next2
                    if o + 2 < O:
                        w1t_next2 = [
                            w1pool.tile([128, H], F32R, tag=f"w1t{kt}", name=f"w1t_{o}_{kt}")
                            for kt in range(KT)
                        ]
                        for kt in range(KT):
                            nc.sync.dma_start(
                                w1t_next2[kt][:],
                                w1_d[kt * 128 : (kt + 1) * 128, o + 2, :],
                            )

                    # matmul1: tmpT[j, x] (6 m-chunks x 6 k-tiles)
                    tmp_all = tmppool.tile([128, KT * L], F32R, tag="tmp", name=f"tmp_{o}")
                    tmp = [tmp_all[:, m * L : (m + 1) * L] for m in range(KT)]
                    for m in range(KT):
                        p1 = psm.tile([128, L], F32, tag="t1", bufs=2)
                        for kt in range(KT):
                            nc.tensor.matmul(
                                p1[:],
                                w1t[kt][:, m * 128 : (m + 1) * 128],
                                xt[kt],
                                start=(kt == 0),
                                stop=(kt == KT - 1),
                            )
                        nc.vector.tensor_copy(tmp[m], p1[:])

                    # matmul2 + linj aug + epilogue per x-chunk
                    p2s = []
                    for c in range(XC):
                        p2 = psm.tile([128, L], F32, tag="t2", bufs=6, name=f"p2_{o}_{c}")
                        p2s.append(p2)
                        nc.tensor.matmul(
                            p2[:],
                            selo[:, o * 128 : (o + 1) * 128],
                            augall[:],
                            start=True,
                            stop=False,
                        )
                    for c in range(XC):
                        p2 = p2s[c]
                        for jr in range(KT):
                            nc.tensor.matmul(
                                p2[:],
                                tmp[jr][:, c * 128 : (c + 1) * 128],
                                xt[jr],
                                start=False,
                                stop=(jr == KT - 1),
                            )
                        osb = outpool.tile([128, L], F32, tag="osb", name=f"osb_{o}_{c}")
                        nc.vector.scalar_tensor_tensor(
                            out=osb[:],
                            in0=p2[:],
                            scalar=linTT[:, c * O + o : c * O + o + 1],
                            in1=csb[c][:],
                            op0=mybir.AluOpType.add,
                            op1=mybir.AluOpType.add,
                        )
                        nc.scalar.dma_start(
                            out_d[o, c * 128 : (c + 1) * 128, :], osb[:]
                        )

    nc.compile()
    return nc


def _get_nc():
    global _cached_nc
    if _cached_nc is None:
        _cached_nc = build_nc()
    return _cached_nc


def _host_consts(weight2):
    w2a = np.zeros((H + 1, 128), dtype=np.float32)
    # cols o: linT' = lin_i + bias; cols O+o: linjT' = lin_j
    w2a[:H, :O] = weight2[:H, :]
    w2a[H, :O] = weight2[2 * H, :]
    w2a[:H, O : 2 * O] = weight2[H : 2 * H, :]
    selo = np.zeros((128, O * 128), dtype=np.float32)
    for o in range(O):
        selo[O + o, o * 128 : (o + 1) * 128] = 1.0
    ident = np.eye(128, dtype=np.float32)
    ones1 = np.ones((1, L), dtype=np.float32)
    tril = np.tril(np.ones((L, L), dtype=np.float32), k=-1)
    c0 = (-NEG * tril - NEG).astype(np.float32)
    return w2a, selo, ident, ones1, c0


def _run(inputs, weight1, weight2, mask, trace=False):
    nc = _get_nc()
    w2a, selo, ident, ones1, c0 = _host_consts(np.asarray(weight2, dtype=np.float32))
    w1 = np.ascontiguousarray(np.asarray(weight1, dtype=np.float32))
    in_maps = []
    for b in range(NCORES):
        m = np.asarray(mask[b], dtype=np.float32)
        in_maps.append(
            {
                "x": np.ascontiguousarray(np.asarray(inputs[b], dtype=np.float32)),
                "w1": w1,
                "w2a": w2a,
                "selo": selo,
                "mrow": np.ascontiguousarray(m[None, :]),
                "ones1": ones1,
                "ident": ident,
                "c0": c0,
            }
        )
    try:
        br = run_bass_kernel_spmd(
            nc, in_maps, core_ids=list(range(NCORES)), trace=trace
        )
        out = np.stack([br.results[b]["out"] for b in range(NCORES)], axis=0)
        return out, br
    except Exception:  # noqa: BLE001
        # Neuron devices occasionally come up wedged from a previous process
        # (NRT_EXEC_UNIT_UNRECOVERABLE). A wedged device recovers on the next
        # fresh process, so retry execution in clean subprocesses.
        out = _run_in_subprocess(in_maps)
        return out, None


def _run_in_subprocess(in_maps):
    import os
    import subprocess
    import sys
    import tempfile
    import time

    d = tempfile.mkdtemp(prefix="biaffine_kernel_")
    inp = os.path.join(d, "in.npz")
    outp = os.path.join(d, "out.npy")
    flat = {}
    for b in range(NCORES):
        for k, v in in_maps[b].items():
            flat[f"{k}__{b}"] = v
    np.savez(inp, **flat)
    runner = os.path.join(d, "runner.py")
    with open(runner, "w") as f:
        f.write(
            f"""
import sys
sys.path.insert(0, {os.path.dirname(os.path.abspath(__file__))!r})
import numpy as np
import kernel
d = np.load({inp!r})
in_maps = [dict() for _ in range(kernel.NCORES)]
for key in d.files:
    k, b = key.rsplit('__', 1)
    in_maps[int(b)][k] = d[key]
nc = kernel._get_nc()
from concourse.bass_utils import run_bass_kernel_spmd
br = run_bass_kernel_spmd(nc, in_maps, core_ids=list(range(kernel.NCORES)), trace=False)
np.save({outp!r}, np.stack([br.results[b]["out"] for b in range(kernel.NCORES)], axis=0))
"""
        )
    last = None
    for attempt in range(4):
        r = subprocess.run(
            [sys.executable, runner], capture_output=True, timeout=1200
        )
        if r.returncode == 0 and os.path.exists(outp):
            return np.load(outp)
        last = r
        time.sleep(5.0)
    raise RuntimeError(
        "device execution failed after retries: "
        + (last.stderr.decode(errors="replace")[-2000:] if last else "")
    )


def kernel(inputs, weight1, weight2, mask):
    out, _ = _run(inputs, weight1, weight2, mask)
    return out
